# revision 34
# baseline (speedup 1.0000x reference)
"""Batch-parallel attention kernel for 8 TRN2 NeuronCores.

Problem: B=16, S=2048, D=128 full (non-causal) attention, fp32 I/O.
Sharding: batch dim across 8 cores (2 batches/core), no collectives.

Per-core layout: everything is computed in "transposed score" space
S^T[k, q] so no on-device transposes are needed:
  - matmul1: S^T[k,q] = (K^T)[d,k]-stationary @ (Q^T)[d,q]-moving,
    contraction over d=128 partitions. Q^T/K^T prepared on host; Q is
    pre-scaled by A' = 128*log2(e)/sqrt(128) so scores arrive in
    "bf16-bit units": true weight = 2^(score/128).
  - exp is split across TWO engines so the PE never starves on it:
      * Act engine: native Exp with scale=ln(2)/128.
      * Vector (DVE): ONE fused 8-stage custom op EXP2_KINK_ANT that
        reads the raw fp32 score straight from PSUM and emits int16
        bf16-bit patterns of ~2^(x/128): r = round128(x) via the
        1.5*2^30 magic add, u = x - r in [-64,64], and a factored
        quadratic (u+P)(B*u+C) + r fitted over the kinked (unshifted)
        window — see the EXPK_* constants.
    Tag-critical late groups are split half/half across both engines
    so their score banks free before the next chunk's m1s.
  - matmul2: out[q, 0:129] = sum_k expS^T[k,q]-stationary @ V_aug[k,:]
    where V_aug = [V | ones]; column 128 accumulates the softmax
    denominator. The 4 q-subtile accumulators are packed 2-per-PSUM
    bank ([128,2,129] tiles); only the first slice of a bank carries
    start=True (bank-wide has_written clear).
  - normalize: DVE reciprocal of the denominator + per-partition
    tensor_scalar multiply, DMA'd out natively. The final chunk's
    normalize+output fans out across DVE/Act and three DMA queues.
  - head: the PE clock ramps over ~3us of continuous work, and the
    first input DMA lands ~11us in (boot + issue + DGE + transfer +
    completion semaphore), so warm-up matmuls on zeroed SBUF burn the
    ramp while the loads fly; batch-0 input DMAs fan out over the
    sync/scalar/gpsimd queues in first-needed order.

PSUM: 3 score tiles x 2 banks + 2 packed accumulator banks = 8 banks.
"""

import math
import os

import ml_dtypes
import numpy as np

import concourse.bass as bass
import concourse.mybir as mybir
import concourse.tile as tile
from concourse import bacc
from concourse.bass_utils import run_bass_kernel_spmd

B, S, D = 16, 2048, 128
N_CORES = 8
BPC = B // N_CORES          # batches per core
DA = D + 1                  # V augmented with ones column
QCHUNK = 512                # q processed per inner pipeline chunk
N_QC = S // QCHUNK          # 4
N_KT = S // 128             # 16 k-tiles
N_G = 8                     # 2-ktile score groups per chunk
SCALE_BITS = 128.0 * math.log2(math.e) / math.sqrt(D)   # Q host prescale
ACT_SCALE = math.log(2.0) / 128.0                       # exp(x*ACT_SCALE) = 2^(x/128)

# Fused single-op Schraudolph constants: out_bits = (u+P)*(B*u+C) + r with
# r = round128(x) via the magic add and u = x - r in [-64, 64]. The window
# is NOT shifted to a clean binade (that would need a 9th ALU stage), so the
# quadratic fits the kinked correction curve; its negative curvature is what
# makes the factored form (u+P)(Bu+C) real-rooted with P*C equal to the
# ~16253-bit constant. L-inf fit: 2.9 bits; centered so the DVE path's mean
# weight error is ~0 (matching the Act path). Max elementwise 2.4%, rms 0.91%.
EXPK_M = float(np.float32(1.5 * 2**30))
EXPK_P = 2371.605167                # imm2: linear-factor offset
EXPK_C = 6.852735                   # s1: quadratic-factor offset
EXPK_B = -0.0024621376              # Src1 tensor fill: quad coefficient

# Exp-engine assignment per group (of 2 k-tiles): DVE takes two full
# groups with the fused 1-op exp; the final group g7 is split half/half
# across Act+DVE so its score banks free within one PE group period.
# This split measured best of the tested Act/DVE balances.
DVE_GROUPS = (2, 6)
SPLIT_GROUPS = (7,)
# score tag per group (round-robin, reuse distance 3)
SCORE_TAG = {0: "sA", 1: "sB", 2: "sD", 3: "sA", 4: "sB", 5: "sA",
             6: "sD", 7: "sB"}
M2_LAG = 4                  # groups of m1/exp emitted ahead of their m2

BF16 = mybir.dt.bfloat16
F32 = mybir.dt.float32
I16 = mybir.dt.int16

TRACE = bool(os.environ.get("BASS_KERNEL_TRACE"))
LAST_RESULTS = None

_CACHE = {}

# PE p-state warm-up: the TRN2 PE clock ramps 0.65 -> 1.2 -> 2.4 GHz over
# ~3us of continuous execution. Keep the PE busy on zeroed SBUF tiles while
# the first input DMAs are in flight so the ramp overlaps the load instead
# of taxing the first real matmuls.
WARM_MM_LONG = 20           # [128,128] warm matmuls (~110ns each at mid clock)
WARM_MM_SHORT = 40          # [128,32] trailing warm matmuls (fine-grained end)


def _register_exp_op():
    """Register the fused Schraudolph exp2-bits op in concourse.dve_ops so
    the per-NEFF DVE table generator can find it. 8 ALU stages reading the
    raw fp32 score from PSUM directly (no separate shift op), sha computed
    at registration so DveOp's pin always matches this build."""
    from concourse import dve_ops
    from concourse.dve_spec import C0, C1, C2, Spec, Src0, Src1
    from concourse.dve_spec import lower as dve_lower
    from concourse.dve_uop import DveOpSpec

    name = "EXP2_KINK_ANT"
    for op in dve_ops.OPS:
        if op.name == name:
            return op

    t = Src0 + C0
    r = t - C0
    u = Src0 - r
    w1 = u + C2
    w3 = (u * Src1) + C1
    body = (w1 * w3) + r

    def ref(in0, in1, s0, s1, imm2):
        x = np.float32(in0)
        tt = np.float32(x + np.float32(s0))
        rr = np.float32(tt - np.float32(s0))
        uu = np.float32(x - rr)
        ww1 = np.float32(uu + np.float32(imm2))
        ww3 = np.float32(np.float32(uu * in1) + np.float32(s1))
        return np.float32(np.float32(ww1 * ww3) + rr)

    spec = Spec(body=body, reference=ref)
    row = max(dve_ops._SUB_OPCODE_FOR_NAME.values()) + 1
    assert row < 0x20
    dve_ops._SUB_OPCODE_FOR_NAME[name] = row
    shas = {}
    for ver in ("v3", "v4"):
        try:
            uops = dve_lower(spec, ver=ver)
            shas[ver] = DveOpSpec(
                name=name, opcode=row, uops=uops, rd1_en=True
            ).sha(ver)
        except Exception:
            pass
    op = dve_ops.DveOp(name, spec, subdim=False, uops_sha=shas)
    dve_ops.OPS.append(op)
    dve_ops.CUSTOM_DVE_SPECS[name] = spec
    return op


def _build():
    exp_op = _register_exp_op()
    nc = bacc.Bacc("TRN2", target_bir_lowering=False, debug=False)

    qT = nc.dram_tensor("qT", [BPC, D, S], BF16, kind="ExternalInput").ap()
    kT = nc.dram_tensor("kT", [BPC, D, S], BF16, kind="ExternalInput").ap()
    # V pre-arranged on host to [BPC, half, partition, ktile, DA] so the
    # load is one fully-contiguous 2064B run per partition (no strided
    # descriptors).
    vA = nc.dram_tensor(
        "vA", [BPC, 2, 128, N_KT // 2, DA], BF16, kind="ExternalInput"
    ).ap()
    out = nc.dram_tensor("out", [BPC, S, D], F32, kind="ExternalOutput").ap()

    with tile.TileContext(nc) as tc:
        with (
            tc.tile_pool(name="qk", bufs=2) as qk_pool,
            tc.tile_pool(name="vp", bufs=2) as v_pool,
            tc.tile_pool(name="warm", bufs=1) as warm_pool,
            tc.tile_pool(name="pexp", bufs=4) as p_pool,
            tc.tile_pool(name="outs", bufs=12) as o_pool,
            tc.tile_pool(name="psum_s", bufs=1, space="PSUM") as psum_s,
            tc.tile_pool(name="psum_acc", bufs=1, space="PSUM") as psum_acc,
        ):
            QS = S // 4
            batch_tiles = {}

            def load_batch(b):
                # Independent DMAs spread across engine queues, first-needed
                # pieces first. The very first matmul only needs kT columns
                # 0:128 (32KB) + the first half of qT quarter 0, so those are
                # split out. For batch 0 the issues fan out over ALL five
                # sequencers (each dma_start costs ~600ns of issuing-engine
                # time, and the sequencers boot staggered: vector ~5.8us,
                # scalar ~5.9, gpsimd ~6.0, sync ~6.8) so the first pieces
                # are in flight as early as possible.
                k0a = qk_pool.tile([128, 128], BF16, tag="kT0a", name="kT0a")
                k0b = qk_pool.tile([128, QS - 128], BF16, tag="kT0b",
                                   name="kT0b")
                kT_rest = [qk_pool.tile([128, QS], BF16, tag=f"kT{h}",
                                        name=f"kT{h}") for h in range(1, 4)]
                v_sb = [v_pool.tile([128, N_KT // 2, DA], BF16, tag=f"v{h}",
                                    name=f"v{h}") for h in range(2)]
                if b == 0:
                    # qT quarter 0 as two half tiles so the first m1 can
                    # start on the first 256 columns.
                    q0a = qk_pool.tile([128, QS // 2], BF16, tag="qT0a",
                                       name="qT0a")
                    q0b = qk_pool.tile([128, QS // 2], BF16, tag="qT0b",
                                       name="qT0b")
                    qT_rest = [qk_pool.tile([128, QS], BF16, tag=f"qT{h}",
                                            name=f"qT{h}") for h in range(1, 4)]
                    qT_sb = [[(q0a, QS // 2), (q0b, QS // 2)]] + [
                        [(t, QS)] for t in qT_rest
                    ]
                    # Only sync/scalar/gpsimd sequencers can issue DMAs.
                    # Sync and scalar reach their first dma_start ~6.6us;
                    # gpsimd's first DIRECT2D lands ~0.6us later. DMA issue
                    # runs ~0.66us on the sequencer, then ~0.65us DGE delay,
                    # the transfer, and a ~0.9us completion-semaphore.
                    nc.scalar.dma_start(out=k0a, in_=kT[b][:, 0:128])
                    nc.sync.dma_start(out=q0a, in_=qT[b][:, 0 : QS // 2])
                    nc.gpsimd.dma_start(out=q0b, in_=qT[b][:, QS // 2 : QS])
                    nc.sync.dma_start(out=k0b, in_=kT[b][:, 128:QS])
                    nc.scalar.dma_start(out=kT_rest[0], in_=kT[b][:, QS : 2 * QS])
                    nc.sync.dma_start(
                        out=kT_rest[1], in_=kT[b][:, 2 * QS : 3 * QS]
                    )
                    nc.scalar.dma_start(out=kT_rest[2], in_=kT[b][:, 3 * QS : S])
                    nc.gpsimd.dma_start(out=v_sb[0], in_=vA[b, 0])
                    nc.gpsimd.dma_start(out=v_sb[1], in_=vA[b, 1])
                    nc.sync.dma_start(out=qT_rest[0],
                                      in_=qT[b][:, QS : 2 * QS])
                    nc.gpsimd.dma_start(out=qT_rest[1],
                                        in_=qT[b][:, 2 * QS : 3 * QS])
                    nc.sync.dma_start(out=qT_rest[2], in_=qT[b][:, 3 * QS : S])
                else:
                    qT_full = [qk_pool.tile([128, QS], BF16, tag=f"qT{h}",
                                            name=f"qT{h}") for h in range(4)]
                    qT_sb = [[(t, QS)] for t in qT_full]
                    nc.sync.dma_start(out=k0a, in_=kT[b][:, 0:128])
                    nc.gpsimd.dma_start(out=qT_full[0], in_=qT[b][:, 0:QS])
                    nc.sync.dma_start(out=k0b, in_=kT[b][:, 128:QS])
                    nc.sync.dma_start(out=kT_rest[0], in_=kT[b][:, QS : 2 * QS])
                    nc.gpsimd.dma_start(out=qT_full[1],
                                        in_=qT[b][:, QS : 2 * QS])
                    nc.gpsimd.dma_start(out=v_sb[0], in_=vA[b, 0])
                    nc.sync.dma_start(
                        out=kT_rest[1], in_=kT[b][:, 2 * QS : 3 * QS]
                    )
                    nc.gpsimd.dma_start(out=qT_full[2],
                                        in_=qT[b][:, 2 * QS : 3 * QS])
                    nc.sync.dma_start(out=kT_rest[2], in_=kT[b][:, 3 * QS : S])
                    nc.gpsimd.dma_start(out=qT_full[3], in_=qT[b][:, 3 * QS : S])
                    nc.gpsimd.dma_start(out=v_sb[1], in_=vA[b, 1])

                def kt_slice(kt):
                    # lhsT AP for k-tile kt given the split first quarter
                    if kt == 0:
                        return k0a
                    if kt < 4:
                        return k0b[:, (kt - 1) * 128 : kt * 128]
                    t = kT_rest[kt // 4 - 1]
                    return t[:, (kt % 4) * 128 : (kt % 4 + 1) * 128]

                batch_tiles[b] = (kt_slice, qT_sb, v_sb)

            def emit_m2(b, qc, g, p_tile, acc):
                _, _, v_sb = batch_tiles[b]
                last = b == BPC - 1 and qc == N_QC - 1
                if last and g == N_G - 1:
                    # Final group of the whole kernel: j-outer in bank order
                    # [0,2,1,3] (j0/j1 share a PSUM bank, as do j2/j3) so
                    # each recip overlaps the OTHER bank's m2 instead of
                    # stalling it; muls split DVE/gpsimd; output DMAs fan
                    # out over sync+scalar — all four tails in parallel.
                    recips = {}
                    for j in (0, 2, 1, 3):
                        for h in range(2):
                            kt = 2 * g + h
                            nc.tensor.matmul(
                                acc[j // 2][:, j % 2, :],
                                lhsT=p_tile[:, h, j * 128 : (j + 1) * 128],
                                rhs=v_sb[kt // 8][:, kt % 8, :],
                                start=False,
                                stop=(kt == N_KT - 1),
                            )
                        a = acc[j // 2][:, j % 2, :]
                        r = o_pool.tile([128, 1], F32, tag="recip",
                                        name="recip")
                        nc.vector.reciprocal(r, a[:, D : D + 1])
                        recips[j] = r
                    osbs = {}
                    for j in (0, 1, 2, 3):
                        a = acc[j // 2][:, j % 2, :]
                        o_sb = o_pool.tile([128, D], F32, tag="o", name="o_sb")
                        if j % 2:
                            # gpsimd can't read PSUM; odd-j muls go to Act
                            # (Copy activation with per-partition scale).
                            nc.scalar.mul(o_sb, a[:, 0:D], recips[j])
                        else:
                            nc.vector.tensor_scalar_mul(o_sb, a[:, 0:D],
                                                        recips[j])
                        osbs[j] = o_sb
                    # No gpsimd issue here: its post-issue DRAIN (~2.5us)
                    # would land on the critical path at kernel end.
                    for j, eng in [(0, nc.sync), (1, nc.scalar),
                                   (2, nc.sync), (3, nc.scalar)]:
                        r0 = qc * QCHUNK + j * 128
                        eng.dma_start(out=out[b, r0 : r0 + 128, :],
                                      in_=osbs[j])
                    return
                for h in range(2):
                    kt = 2 * g + h
                    for j in range(4):
                        # acc[j//2][:, j%2, :] packs two q-subtiles per PSUM
                        # bank; start=True clears has_written for the WHOLE
                        # bank, so only the first slice (j even) may carry it.
                        nc.tensor.matmul(
                            acc[j // 2][:, j % 2, :],
                            lhsT=p_tile[:, h, j * 128 : (j + 1) * 128],
                            rhs=v_sb[kt // 8][:, kt % 8, :],
                            start=(kt == 0 and j % 2 == 0),
                            stop=(kt == N_KT - 1),
                        )
                if g == N_G - 1:
                    for j in range(4):
                        emit_normalize_j(b, qc, acc, j)

            def emit_normalize_j(b, qc, acc, j):
                a = acc[j // 2][:, j % 2, :]
                recip = o_pool.tile([128, 1], F32, tag="recip",
                                    name="recip")
                nc.vector.reciprocal(recip, a[:, D : D + 1])
                o_sb = o_pool.tile([128, D], F32, tag="o", name="o_sb")
                nc.vector.tensor_scalar_mul(o_sb, a[:, 0:D], recip)
                r0 = qc * QCHUNK + j * 128
                nc.sync.dma_start(out=out[b, r0 : r0 + 128, :], in_=o_sb)

            # One continuous software pipeline across every (batch, q-chunk,
            # k-group): m2 for group g is emitted M2_LAG groups after its m1,
            # so the in-order PE queue always has independent m1 work while
            # the two exp engines run, with no drain at chunk boundaries.
            # Pull the ~1.3us exp table load to t=0 so it overlaps the input
            # DMAs instead of stalling the first real exp.
            wtile = warm_pool.tile([128, 1], F32)
            nc.vector.memset(wtile, 0.0)
            wz = warm_pool.tile([128, 128], BF16, name="warm_z")
            nc.vector.memset(wz, 0.0)
            nc.scalar.activation(
                wtile, wtile, mybir.ActivationFunctionType.Exp
            )

            pending = []
            load_batch(0)

            # Full-shape constant: the quadratic coefficient b for the
            # custom DVE op's Src1 slot. [P,1] broadcast Src1 faults on HW
            # (bisected); a full elementwise tensor works, and 2 dims keeps
            # the TTSS struct so imm2 stays available. Emitted after batch
            # 0's vector-queue DMA issues (not needed until the first DVE
            # exp group, ~4us later).
            bconst = warm_pool.tile([128, 2 * QCHUNK], F32, name="bconst")
            nc.vector.memset(bconst, EXPK_B)

            # PE p-state warm-up: matmuls on the zeroed tile into the sA
            # score bank while the first input DMAs are in flight. The PE
            # clock ramps 0.65 -> 1.2 -> 2.4 GHz over ~3us of continuous
            # work, so burn the ramp on junk instead of the first real m1s.
            s_warm = psum_s.tile(
                [128, 2, QCHUNK], F32, tag=SCORE_TAG[0], name=SCORE_TAG[0]
            )
            for i in range(WARM_MM_LONG):
                nc.tensor.matmul(
                    s_warm[:, 0, 0:128], lhsT=wz, rhs=wz,
                    start=True, stop=True,
                )
            for i in range(WARM_MM_SHORT):
                nc.tensor.matmul(
                    s_warm[:, 0, 0:32], lhsT=wz, rhs=wz[:, 0:32],
                    start=True, stop=True,
                )
            for b in range(BPC):
                for qc in range(N_QC):
                    kt_slice, qT_sb, _ = batch_tiles[b]
                    last_chunk = b == BPC - 1 and qc == N_QC - 1
                    acc = [
                        psum_acc.tile(
                            [128, 2, DA], F32, tag=f"acc{i}", name=f"acc{i}"
                        )
                        for i in range(2)
                    ]
                    for g in range(N_G):
                        # Chunk 0 has no m2 backlog: its m1 burst outruns
                        # either exp engine's serial queue. Alternating
                        # full groups between Act and DVE halves each
                        # queue's depth so the score tags recycle in time.
                        if b == 0 and qc == 0:
                            use_dve = g in (1, 3, 5)
                        else:
                            use_dve = g in DVE_GROUPS
                        s_psum = psum_s.tile(
                            [128, 2, QCHUNK], F32, tag=SCORE_TAG[g],
                            name=SCORE_TAG[g],
                        )
                        for h in range(2):
                            kt = 2 * g + h
                            col = 0
                            for qt, w in qT_sb[qc]:
                                nc.tensor.matmul(
                                    s_psum[:, h, col : col + w],
                                    lhsT=kt_slice(kt),
                                    rhs=qt,
                                    start=True,
                                    stop=True,
                                )
                                col += w
                        p_tile = p_pool.tile(
                            [128, 2, QCHUNK], BF16, tag=f"p{g % 2}",
                            name=f"p{g % 2}",
                        )
                        if use_dve:
                            nc.vector._custom_dve(
                                exp_op,
                                out=p_tile.bitcast(I16),
                                in0=s_psum,
                                in1=bconst,
                                s0=EXPK_M,
                                s1=EXPK_C,
                                imm2=EXPK_P,
                            )
                        elif g in SPLIT_GROUPS:
                            # Split exp across both engines so the score
                            # banks free within one group period (a full-
                            # tile exp on either engine is slower than the
                            # PE's per-group cadence).
                            nc.scalar.activation(
                                p_tile[:, 0, :],
                                s_psum[:, 0, :],
                                mybir.ActivationFunctionType.Exp,
                                scale=ACT_SCALE,
                            )
                            nc.vector._custom_dve(
                                exp_op,
                                out=p_tile.bitcast(I16)[:, 1, :],
                                in0=s_psum[:, 1, :],
                                in1=bconst[:, 0:QCHUNK],
                                s0=EXPK_M,
                                s1=EXPK_C,
                                imm2=EXPK_P,
                            )
                        else:
                            nc.scalar.activation(
                                p_tile,
                                s_psum,
                                mybir.ActivationFunctionType.Exp,
                                scale=ACT_SCALE,
                            )
                        pending.append((b, qc, g, p_tile, acc))
                        if len(pending) > M2_LAG:
                            emit_m2(*pending.pop(0))
                        # prefetch next batch's inputs once this batch's
                        # second q-chunk is underway
                        if b + 1 < BPC and qc == 1 and g == 3:
                            load_batch(b + 1)
                    # Flush the chunk: puts this chunk's normalize (which
                    # frees the acc PSUM banks) AHEAD of the next chunk's
                    # exp work in the DVE queue, so the next chunk's first
                    # m2 group never stalls on the accumulator banks.
                    # (A partial drain that leaves the PE's OOO window
                    # unclogged trades the ~390ns boundary m1 stall for
                    # slower m2s — measured net-neutral, so keep this.)
                    while pending:
                        emit_m2(*pending.pop(0))

    nc.compile()
    return nc


def _get_nc():
    if "nc" not in _CACHE:
        _CACHE["nc"] = _build()
    return _CACHE["nc"]


def kernel(query, key, value):
    global LAST_RESULTS
    bf16 = ml_dtypes.bfloat16
    q = np.ascontiguousarray(
        (np.asarray(query, dtype=np.float32) * np.float32(SCALE_BITS))
        .transpose(0, 2, 1)
    ).astype(bf16)
    k = np.ascontiguousarray(
        np.asarray(key, dtype=np.float32).transpose(0, 2, 1)
    ).astype(bf16)
    v = np.asarray(value, dtype=np.float32)
    v_aug = np.concatenate(
        [v, np.ones((B, S, 1), dtype=np.float32)], axis=2
    ).astype(bf16)
    # [B, S, DA] -> [B, half, partition, ktile, DA] so the device load is
    # one contiguous run per partition.
    v_aug = np.ascontiguousarray(
        v_aug.reshape(B, 2, N_KT // 2, 128, DA).transpose(0, 1, 3, 2, 4)
    )

    nc = _get_nc()
    in_maps = [
        {
            "qT": q[i * BPC : (i + 1) * BPC],
            "kT": k[i * BPC : (i + 1) * BPC],
            "vA": v_aug[i * BPC : (i + 1) * BPC],
        }
        for i in range(N_CORES)
    ]
    res = run_bass_kernel_spmd(
        nc, in_maps, core_ids=list(range(N_CORES)), trace=TRACE
    )
    LAST_RESULTS = res
    out = np.empty((B, S, D), dtype=np.float32)
    for i in range(N_CORES):
        out[i * BPC : (i + 1) * BPC] = res.results[i]["out"]
    return out

